# revision 12
# baseline (speedup 1.0000x reference)
"""Causal single-head attention (B=8, T=4096, C=1024, H=128) on 8 TRN2
NeuronCores, data-parallel over batch: core b computes batch element b.

Per core: x [T, C] f32 and Wq/Wk/Wv [C, H] f32 (replicated) -> out [T, H] f32.

v2 kernel: software-pipelined Tile program, fp16 compute / fp32 psum.
Structure per 512-row chunk c (t0 = 512c):
  Phase A (projections, interleaved into phase B of chunk c-1):
    SWDGE cast-DMA loads x chunk as fp16, xbar-DMA transposes to xT
    (contraction dim on partitions), PE computes qT/kT [H, 512] (w stationary,
    N=512 streams) and v [512, H] (xT blocks stationary, N=128).
  Phase B (attention):
    Off-diagonal key blocks are processed in PAIRS: two score matmuls
    sT[j, q] = kT_j.T @ qT into a 2-bank PSUM tile, ONE activation does
    exp on both (halves Act-engine instruction overhead), then the 8 PV
    matmuls for the PREVIOUS pair (1-pair lag hides Act latency from PE).
    Diagonal blocks: per-j exp on the causally needed q range + triangular
    mask multiply. PV accumulates out_psum[q, H+1] += P_j.T @ [v_j | 1]
    (ones column yields softmax denominators). DVE normalizes, DVE-queue
    HWDGE stores the result.
Projection matmuls + casts of chunk c+1 and the x load of chunk c+2 are
emitted in slots between pairs so the PE never idles while Act works.
"""
import numpy as np

import concourse.bass as bass
import concourse.mybir as mybir
import concourse.tile as tile
from concourse.bass import ts
from contextlib import ExitStack

F16 = mybir.dt.float16
F32 = mybir.dt.float32

B, T, C, H = 8, 4096, 1024, 128

# ---------------------------------------------------------------------------
# Workaround for the walrus build in this container: each TPB instruction may
# carry at most ONE sync-wait ("Too many sync wait commands" otherwise), but
# Tile attaches several. Keep only the last wait per instruction and hoist the
# others onto preceding same-engine NoOps (engines execute their stream in
# order, so the gating semantics are identical). The tail drain gets the same
# treatment.
# ---------------------------------------------------------------------------
_MAX_WAITS = 1
_orig_add_instruction = tile.TileContext._add_instruction


def _split_waits_add_instruction(self, inst):
    si = inst.sync_info
    if (
        si is not None
        and len(si.on_wait) > _MAX_WAITS
        and inst.engine != mybir.EngineType.Unassigned
    ):
        waits = list(si.on_wait)
        extra, keep = waits[:-_MAX_WAITS], waits[-_MAX_WAITS:]
        for w in extra:
            nop = mybir.InstNoOp(
                name=self.nc.get_next_instruction_name(),
                engine=inst.engine,
                ins=[],
                outs=[],
                bass_nofuse=True,
                sync_info=mybir.SyncInfo(on_wait=[w], on_update=[]),
                debug=inst.debug,
            )
            _orig_add_instruction(self, nop)
        inst.sync_info = mybir.SyncInfo(on_wait=keep, on_update=list(si.on_update))
    return _orig_add_instruction(self, inst)


def _split_drain_and_barrier(self, tick_clock, wait_clock):
    nc = self.nc
    probe = nc.sync.nop(nofuse=True, hint="tile_drain_wait_split")
    wait_clock.add_sem_waits(
        probe.ins, tile.ScopedClock({None: tick_clock.global_clock})
    )
    si = probe.ins.sync_info
    waits = list(si.on_wait) if si is not None else []
    if len(waits) > _MAX_WAITS:
        probe.ins.sync_info = mybir.SyncInfo(
            on_wait=waits[:_MAX_WAITS], on_update=list(si.on_update)
        )
        rest = waits[_MAX_WAITS:]
        for i in range(0, len(rest), _MAX_WAITS):
            extra = nc.sync.nop(nofuse=True, hint=f"tile_drain_wait_split_{i}")
            extra.ins.sync_info = mybir.SyncInfo(
                on_wait=rest[i : i + _MAX_WAITS], on_update=[]
            )
    nc.sync.drain()
    nc.all_engine_barrier()
    assert self.sems is not None
    popped = nc._tile_sem_poison_stack.pop()
    assert popped is self._sem_poison
    nc.clear_and_free_semaphores(list(self.sems.allocated().values()))
    nc.all_engine_barrier()


def _apply_tile_patch():
    tile.TileContext._drain_and_barrier = _split_drain_and_barrier
    tile.TileContext._add_instruction = _split_waits_add_instruction


# ---------------------------------------------------------------------------
# Kernel builder
# ---------------------------------------------------------------------------
def build_attention(dtype=F16):
    TB = T // 128   # 32 key blocks
    CB = C // 128   # 8 contraction blocks
    NCH = T // 512  # 8 chunks
    scale = float(H) ** -0.5

    nc = bass.Bass()
    x = nc.dram_tensor("x", [T, C], F32, kind="ExternalInput")
    wq = nc.dram_tensor("wq", [C, H], F32, kind="ExternalInput")
    wk = nc.dram_tensor("wk", [C, H], F32, kind="ExternalInput")
    wv = nc.dram_tensor("wv", [C, H], F32, kind="ExternalInput")
    out = nc.dram_tensor("out", [T, H], F32, kind="ExternalOutput")

    with tile.TileContext(nc) as tc, ExitStack() as ctx:
        const = ctx.enter_context(tc.tile_pool(name="const", bufs=1))
        xsb = ctx.enter_context(tc.tile_pool(name="xsb", bufs=2))
        xtsb = ctx.enter_context(tc.tile_pool(name="xtsb", bufs=2))
        persist = ctx.enter_context(tc.tile_pool(name="persist", bufs=1))
        pP = ctx.enter_context(tc.tile_pool(name="pP", bufs=4))
        pQ = ctx.enter_context(tc.tile_pool(name="pQ", bufs=2))
        osb = ctx.enter_context(tc.tile_pool(name="osb", bufs=4))
        # PSUM: sc 2x2 banks + ops 2 banks + mm 1 bank + pv 1 bank = 8
        ps_sc = ctx.enter_context(tc.tile_pool(name="ps_sc", bufs=2, space="PSUM"))
        ps_ops = ctx.enter_context(tc.tile_pool(name="ps_ops", bufs=1, space="PSUM"))
        ps_mm = ctx.enter_context(tc.tile_pool(name="ps_mm", bufs=1, space="PSUM"))
        ps_pv = ctx.enter_context(tc.tile_pool(name="ps_pv", bufs=1, space="PSUM"))

        # --- constants / persistent state -------------------------------
        w16 = {}
        for name in ("q", "k", "v"):
            w16[name] = const.tile(
                [128, CB, H], dtype, tag=f"w{name}", name=f"w16{name}"
            )
        # first chunk loaded per-tb so transposes can start early
        x16_first = xsb.tile([128, 4, C], dtype, tag="x16", name="x16_c0")
        nc.gpsimd.dma_start(x16_first[:, 0, :], x[0:128, :])
        nc.gpsimd.dma_start(
            w16["q"][:], wq[:].rearrange("(cb ci) h -> ci cb h", ci=128)
        )
        for tb in range(1, 4):
            nc.gpsimd.dma_start(
                x16_first[:, tb, :], x[tb * 128 : (tb + 1) * 128, :]
            )
        nc.gpsimd.dma_start(
            w16["k"][:], wk[:].rearrange("(cb ci) h -> ci cb h", ci=128)
        )
        nc.gpsimd.dma_start(
            w16["v"][:], wv[:].rearrange("(cb ci) h -> ci cb h", ci=128)
        )
        # mask16[jl, ql] = 1 if ql >= jl else 0 (transposed-score layout)
        mask16 = const.tile([128, 128], dtype, tag="mask")
        nc.gpsimd.memset(mask16[:], 1.0)
        nc.gpsimd.affine_select(
            out=mask16[:], in_=mask16[:],
            compare_op=mybir.AluOpType.is_ge,
            fill=0.0, base=0, pattern=[[1, 128]], channel_multiplier=-1,
        )

        qT16 = persist.tile([128, T], dtype, tag="qT")
        kT16 = persist.tile([128, T], dtype, tag="kT")
        v16 = persist.tile([128, TB, H + 1], dtype, tag="v")
        nc.vector.memset(v16[:], 1.0)  # ones column survives in col H

        x16_t = {0: x16_first}

        # --- phase-A op generators --------------------------------------
        def a_load(cc):
            def op():
                x16_t[cc] = xsb.tile(
                    [128, 4, C], dtype, tag="x16", name=f"x16_c{cc}"
                )
                nc.gpsimd.dma_start(
                    x16_t[cc][:],
                    x[cc * 512 : (cc + 1) * 512, :].rearrange(
                        "(tb ti) c -> ti tb c", ti=128
                    ),
                )
            return op

        xt16_t = {}

        def a_transpose(cc, tb):
            def op():
                if tb == 0:
                    xt16_t[cc] = xtsb.tile(
                        [128, CB, 512], dtype, tag="xt16", name=f"xt16_c{cc}"
                    )
                nc.sync.dma_start(
                    xt16_t[cc][:, :, ts(tb, 128)], x16_t[cc][:, tb, :],
                    transpose=True,
                )
            return op

        proj_mm = {}

        def a_proj_qk(cc, name, cb):
            def op():
                if cb == 0:
                    proj_mm[(cc, name)] = ps_mm.tile(
                        [128, 512], F32, tag="mm", name=f"mm_{name}_{cc}"
                    )
                nc.tensor.matmul(
                    proj_mm[(cc, name)][:], w16[name][:, cb, :],
                    xt16_t[cc][:, cb, :],
                    start=(cb == 0), stop=(cb == CB - 1),
                )
            return op

        def a_cast_qk(cc, name, dstT):
            def op():
                nc.vector.tensor_copy(
                    dstT[:, cc * 512 : (cc + 1) * 512], proj_mm[(cc, name)][:]
                )
            return op

        def a_proj_v(cc, tb, cb):
            def op():
                if tb == 0 and cb == 0:
                    proj_mm[(cc, "v")] = ps_pv.tile(
                        [128, 4, 128], F32, tag="pv", name=f"pv_{cc}"
                    )
                # pv bank hosts 4 sequential accumulation regions (one per
                # tb). Only the first matmul of the bank uses start=True:
                # its lazy-zero marks the whole bank, so each later tb's
                # first write (start=False onto pending-zero bytes)
                # overwrites instead of accumulating. stop on the last.
                nc.tensor.matmul(
                    proj_mm[(cc, "v")][:, tb, :],
                    xt16_t[cc][:, cb, ts(tb, 128)], w16["v"][:, cb, :],
                    start=(cb == 0 and tb == 0),
                    stop=(cb == CB - 1 and tb == 3),
                )
            return op

        def a_cast_v(cc):
            def op():
                nc.vector.tensor_copy(
                    v16[:, cc * 4 : cc * 4 + 4, 0:H], proj_mm[(cc, "v")][:]
                )
            return op

        def phase_a_ops(cc):
            """Ops to emit during phase B of chunk cc-1 (loads handled
            separately, two chunks ahead)."""
            ops = []
            for tb in range(4):
                ops.append(a_transpose(cc, tb))
            for cb in range(CB):
                ops.append(a_proj_qk(cc, "q", cb))
            ops.append(a_cast_qk(cc, "q", qT16))
            for cb in range(CB):
                ops.append(a_proj_qk(cc, "k", cb))
            ops.append(a_cast_qk(cc, "k", kT16))
            for tb in range(4):
                for cb in range(CB):
                    ops.append(a_proj_v(cc, tb, cb))
            ops.append(a_cast_v(cc))
            return ops

        # --- prologue: chunk 0 phase A + chunk 1 load --------------------
        for op in phase_a_ops(0):
            op()
        a_load(1)()

        # --- main loop ---------------------------------------------------
        for c in range(NCH):
            t0 = c * 512
            a_ops = phase_a_ops(c + 1) if c + 1 < NCH else []
            if c + 2 < NCH:
                a_ops.insert(4, a_load(c + 2))
            npairs = 2 * c          # off-diagonal pairs
            slots = npairs + 4      # pair slots + diag-region slots
            a_per_slot = -(-len(a_ops) // slots) if a_ops else 0
            a_i = 0

            def emit_a_slot():
                nonlocal a_i
                for _ in range(a_per_slot):
                    if a_i < len(a_ops):
                        a_ops[a_i]()
                        a_i += 1

            ops01 = ps_ops.tile([128, 2, H + 1], F32, tag="o01", name=f"o01_{c}")
            ops23 = ps_ops.tile([128, 2, H + 1], F32, tag="o23", name=f"o23_{c}")
            opsr = [
                ops01[:, 0, :], ops01[:, 1, :], ops23[:, 0, :], ops23[:, 1, :]
            ]

            def emit_pv(p16pair, m):
                """PV matmuls for off-diagonal pair m (j = 2m, 2m+1).

                Each ops bank hosts TWO accumulation regions (qb even/odd).
                Only the even region's first matmul starts the bank (its
                lazy-zero marks both regions); the odd region's first write
                lands on pending-zero bytes and overwrites."""
                for qb in range(4):
                    for jt in range(2):
                        j = 2 * m + jt
                        nc.tensor.matmul(
                            opsr[qb], p16pair[:, jt, ts(qb, 128)],
                            v16[:, j, :],
                            start=(j == 0 and qb % 2 == 0), stop=False,
                        )

            # ---- off-diagonal pairs, PV lagging by one pair ----
            prev = None
            for m in range(npairs):
                sc = ps_sc.tile([128, 2, 512], F32, tag="sc", name=f"sc_{c}_{m}")
                for jt in range(2):
                    nc.tensor.matmul(
                        sc[:, jt, :], kT16[:, ts(2 * m + jt, 128)],
                        qT16[:, t0 : t0 + 512],
                        start=True, stop=True,
                    )
                p16 = pP.tile([128, 2, 512], dtype, tag="p", name=f"p_{c}_{m}")
                nc.scalar.activation(
                    p16[:], sc[:],
                    mybir.ActivationFunctionType.Exp, scale=scale,
                )
                emit_a_slot()
                if prev is not None:
                    emit_pv(*prev)
                prev = (p16, m)

            # ---- diagonal blocks j = 4c+d ----
            pq = pQ.tile([128, 4, 512], dtype, tag="pq", name=f"pq_{c}")
            scA = ps_sc.tile([128, 2, 512], F32, tag="sc", name=f"scA_{c}")
            nc.tensor.matmul(
                scA[:, 0, :], kT16[:, ts(4 * c, 128)], qT16[:, t0 : t0 + 512],
                start=True, stop=True,
            )
            nc.tensor.matmul(
                scA[:, 1, 128:512], kT16[:, ts(4 * c + 1, 128)],
                qT16[:, t0 + 128 : t0 + 512],
                start=True, stop=True,
            )
            if prev is not None:
                emit_pv(*prev)
                prev = None
            for d in range(2):
                q_lo = d * 128
                nc.scalar.activation(
                    pq[:, d, q_lo:512], scA[:, d, q_lo:512],
                    mybir.ActivationFunctionType.Exp, scale=scale,
                )
                nc.vector.tensor_mul(
                    pq[:, d, ts(d, 128)], pq[:, d, ts(d, 128)], mask16[:]
                )
            emit_a_slot()
            scB = ps_sc.tile([128, 2, 512], F32, tag="sc", name=f"scB_{c}")
            for i, d in enumerate((2, 3)):
                q_lo = d * 128
                nc.tensor.matmul(
                    scB[:, i, q_lo:512], kT16[:, ts(4 * c + d, 128)],
                    qT16[:, t0 + q_lo : t0 + 512],
                    start=True, stop=True,
                )
            for i, d in enumerate((2, 3)):
                q_lo = d * 128
                nc.scalar.activation(
                    pq[:, d, q_lo:512], scB[:, i, q_lo:512],
                    mybir.ActivationFunctionType.Exp, scale=scale,
                )
                nc.vector.tensor_mul(
                    pq[:, d, ts(d, 128)], pq[:, d, ts(d, 128)], mask16[:]
                )
            # diagonal PV (all query blocks; banks close at the odd
            # region's final matmul), then normalize + store
            for qb in range(4):
                for d in range(qb + 1):
                    nc.tensor.matmul(
                        opsr[qb], pq[:, d, ts(qb, 128)], v16[:, 4 * c + d, :],
                        start=(c == 0 and d == 0 and qb % 2 == 0),
                        # stop on the bank's final matmul (odd region's last)
                        stop=(d == qb and qb % 2 == 1),
                    )
                if qb % 2 == 1:
                    emit_a_slot()
            for qb in range(4):
                rec = osb.tile([128, 1], F32, tag="rec")
                nc.vector.reciprocal(rec[:], opsr[qb][:, H : H + 1])
                o32 = osb.tile([128, H], F32, tag="o32")
                nc.vector.tensor_scalar_mul(o32[:], opsr[qb][:, 0:H], rec[:])
                nc.gpsimd.dma_start(
                    out[t0 + qb * 128 : t0 + (qb + 1) * 128, :], o32[:]
                )
            # flush any remaining phase-A ops
            while a_i < len(a_ops):
                a_ops[a_i]()
                a_i += 1

    return nc


_NC_CACHE = None


def _get_nc():
    global _NC_CACHE
    if _NC_CACHE is None:
        _apply_tile_patch()
        _NC_CACHE = build_attention()
    return _NC_CACHE


def _install_ntff_hook_shim():
    """antenv.axon_hooks is absent on this image, which makes
    run_bass_kernel_spmd(trace=True) crash instead of degrading. Provide the
    module and register the ctypes NTFF hook the boot script would have."""
    import sys, types
    try:
        import antenv.axon_hooks  # noqa: F401
        return
    except ImportError:
        pass
    try:
        import antenv
    except ImportError:
        return
    mod = types.ModuleType("antenv.axon_hooks")
    _hook = [None]
    mod.set_axon_ntff_profile_hook = lambda h: _hook.__setitem__(0, h)
    mod.get_axon_ntff_profile_hook = lambda: _hook[0]
    sys.modules["antenv.axon_hooks"] = mod
    antenv.axon_hooks = mod
    try:
        from trn_agent_boot.trn_boot import _ntff_profile_via_ctypes
        mod.set_axon_ntff_profile_hook(
            _ntff_profile_via_ctypes("/opt/axon/libaxon_pjrt.so")
        )
    except Exception:
        pass


def kernel(x, Wk, Wq, Wv, trace=False):
    """Full inputs in, full output out. Shards batch across the 8 cores."""
    from concourse.bass_utils import run_bass_kernel_spmd

    if trace:
        _install_ntff_hook_shim()

    x = np.ascontiguousarray(np.asarray(x, dtype=np.float32))
    Wk = np.ascontiguousarray(np.asarray(Wk, dtype=np.float32))
    Wq = np.ascontiguousarray(np.asarray(Wq, dtype=np.float32))
    Wv = np.ascontiguousarray(np.asarray(Wv, dtype=np.float32))
    assert x.shape == (B, T, C), x.shape

    nc = _get_nc()
    in_maps = [
        {"x": x[b], "wq": Wq, "wk": Wk, "wv": Wv} for b in range(B)
    ]
    res = run_bass_kernel_spmd(nc, in_maps, core_ids=list(range(B)), trace=trace)
    outp = np.stack([res.results[b]["out"] for b in range(B)], axis=0)
    if trace:
        return outp, res.exec_time_ns
    return outp
